# revision 7
# baseline (speedup 1.0000x reference)
"""AttentionMixerRec Trainium2 kernel, v2.

scores[b,h,l,s] = qt[b,h,l,:] . emb[b,s,:]
  qt = (sum_{j<=l} vm[b,j] ed[b,j]) @ lin_l^T @ (WQ_h^T WK_h) / sqrt(D)
pooled = mean_h (sum_l softmax(scores)^4)^(1/4);  out = pooled @ emb

Key structure vs v1: per-core dedup'd bf16 table in HBM; bulk SWDGE
dma_gather (single_packet=False) delivers emb both transposed [d,(b,s)]
(scores rhs) and plain [s,(b,d)] (final contraction lhsT) in 4 big
instructions; the masked cumsum over L rides PSUM accumulation; exp is
group-batched (8 ACT ops). Data-parallel over B across 8 cores.
"""
import sys
for _p in ('/opt/trn_rl_repo',):
    if _p not in sys.path:
        sys.path.append(_p)   # fallback only; prefer the axon-site copy

import numpy as np
import ml_dtypes
ml_bf16 = ml_dtypes.bfloat16
from concourse import bass, bacc, mybir
from concourse.tile import TileContext
from concourse.masks import make_identity
from concourse.bass_utils import run_bass_kernel_spmd

B, S, D, V, L, H = 256, 200, 256, 100000, 5, 4
NCORES = 8
BL = B // NCORES            # 32 batch rows per core
S0, S1 = 128, S - 128       # seq split 128 + 72
NR = H * L                  # 20
GS = 4                      # batch rows per group
NG = BL // GS               # 8 groups
NT = BL * S                 # compact table rows (>= unique count)
NH = BL // 2                # embT half: 16 b's
NIT = NH * S                # 3200 idxs per embT half
NQ = 8 * S                  # 1600 real idxs per embT quarter (8 b's)
NQI = 1664                  # padded to a multiple of 128
F32 = mybir.dt.float32
BF16 = mybir.dt.bfloat16
I16 = mybir.dt.int16
I32 = mybir.dt.int32

_CACHED = {}


def _patched_act_tables(arch):
    """Steer the act-table-load pass to the one set holding BOTH Exp and Ln
    (natural_log_exp_and_others) so the Exp<->Ln alternation needs a single
    LoadActFuncSet instead of one 1.28us reload per switch. Set order (and
    thus the emitted act_func_set_id) is unchanged; we only hide exp/ln
    from the other sets so the chooser can't pick them."""
    import concourse.hw_specs as _hw
    t = _hw.get_activation_tables(arch)
    exp = mybir.ActivationFunctionType.Exp
    ln = mybir.ActivationFunctionType.Ln
    both = t.get('natural_log_exp_and_others')
    if both is not None and exp in both and ln in both:
        for name, s in t.items():
            if name != 'natural_log_exp_and_others':
                s.discard(exp)
                s.discard(ln)
    return t


def _build(reps=1):
    import concourse.bacc as _bacc_mod
    _orig_tables = _bacc_mod.get_activation_tables
    _bacc_mod.get_activation_tables = _patched_act_tables
    try:
        return _build_inner(reps)
    finally:
        _bacc_mod.get_activation_tables = _orig_tables


def _build_inner(reps=1):
    nc = bacc.Bacc()
    ctable = nc.declare_dram_parameter("ctable", [NT, D], BF16, isOutput=False)
    # packed indexes: idx_small = [ed | e4 | vmf(f32) | vm4(f32)] as i32;
    # idx_big = [embT quarters (4x104) | lo (256) | hi (256)] as i16
    idx_small = nc.declare_dram_parameter("idx_small", [128, 8], I32, isOutput=False)
    idx_big = nc.declare_dram_parameter("idx_big", [128, 2 * (NIT // 16) + 512 + GS * H],
                                        I16, isOutput=False)
    w_d = nc.declare_dram_parameter("w_d", [(L + H) * 2 * 128, 256], BF16,
                                    isOutput=False)
    out = nc.declare_dram_parameter("out", [128, 2 * BL], F32, isOutput=True)

    with TileContext(nc) as tc:
        with tc.tile_pool(name="sb", bufs=1) as sb, \
             tc.tile_pool(name="sbg", bufs=6) as sbg, \
             tc.tile_pool(name="ps_t", bufs=1, space="PSUM") as ps_t, \
             tc.tile_pool(name="ps_sp", bufs=2, space="PSUM") as ps_sp, \
             tc.tile_pool(name="ps_p4", bufs=3, space="PSUM") as ps_p4, \
             tc.tile_pool(name="ps_op", bufs=1, space="PSUM") as ps_op:

            for _rep in range(reps):
                ident = sb.tile([128, 128], F32, tag="ident")
                make_identity(nc, ident[:])

                # ---- input loads (HWDGE) ----
                ism_sb = sb.tile([128, 8], I32, tag="ism")
                ibg_sb = sb.tile([128, 2 * (NIT // 16) + 512 + GS * H], I16, tag="ibg")
                ied_sb = ism_sb[:, 0:1]
                ie4_sb = ism_sb[0:BL, 1:2]
                vmf_sb = ism_sb[:, 2:3].bitcast(F32)
                vm4_sb = ism_sb[0:BL, 3:4].bitcast(F32)
                NTC = 2 * (NIT // 16)
                itq_sb = ibg_sb[:, 0:NTC]
                ilo_sb = ibg_sb[:, NTC:NTC + 256]
                ihi_sb = ibg_sb[:, NTC + 256:NTC + 512]
                hsel_v = ibg_sb[:, NTC + 512:NTC + 512 + GS * H].bitcast(BF16)
                bsel_v = ism_sb[0:GS * H, 4:8].bitcast(F32)
                w_sb = sb.tile([128, (L + H) * 2 * 256], BF16, tag="w")
                lin_sb = w_sb[:, 0:L * 2 * 256]
                m_sb = w_sb[:, L * 2 * 256:]

                # ed-gather indexes first (tiny; they gate the q-chain),
                # then the big gather indexes
                nc.sync.dma_start(out=ism_sb[:], in_=idx_small[:])
                nc.sync.dma_start(out=ibg_sb[:], in_=idx_big[:])
                nc.sync.dma_start(
                    out=w_sb[:].rearrange("p (k c) -> p k c", c=256),
                    in_=w_d[:].rearrange("(k p) c -> p k c", p=128))


                # ---- gathers (SWDGE) ----
                # ed rows first (tiny transfer, gates the whole q-chain):
                # partition 32*l+b holds emb[edidx[b,l]] for l<4; second
                # tile covers l=4.
                ed_sb = sb.tile([128, D], BF16, tag="ed")
                e4_sb = sb.tile([BL, D], BF16, tag="e4")
                nc.gpsimd.indirect_dma_start(
                    out=ed_sb[:], out_offset=None, in_=ctable[:],
                    in_offset=bass.IndirectOffsetOnAxis(ap=ied_sb, axis=0))
                nc.gpsimd.indirect_dma_start(
                    out=e4_sb[:], out_offset=None, in_=ctable[:],
                    in_offset=bass.IndirectOffsetOnAxis(ap=ie4_sb, axis=0))

                # embT halves [128, 2, NIT] (16 b's each, no padding);
                # embT[p, dc, i] = ctable[id(b,s), dc*128+p], i = (b%16)*S + s.
                # half b is issued after embS so the LAST DMA is the one the
                # late score matmuls genuinely need (dodges sem-lane-aliased
                # false waits of the pool stages on embS completions).
                embT = [sb.tile([128, 2 * NIT], BF16, tag=f"embT{k}",
                                name=f"embT{k}") for k in range(2)]

                def emit_embT(k):
                    nc.gpsimd.dma_gather(
                        embT[k][:].rearrange("p (c i) -> p c i", c=2),
                        ctable[:],
                        itq_sb[:, k * (NIT // 16):(k + 1) * (NIT // 16)],
                        NIT, NIT, D, transpose=True, single_packet=False)

                emit_embT(0)
                emit_embT(1)

                # embS: lo [s<128 on partitions, b, d]; hi [s-128, b, d]
                # (hi rows 72..127 are dummy gathers of row 0)
                emb_lo = sb.tile([128, BL * D], BF16, tag="emblo")
                emb_hi = sb.tile([128, BL * D], BF16, tag="embhi")
                nc.gpsimd.dma_gather(
                    emb_lo[:].rearrange("p (c e) -> p c e", e=D),
                    ctable[:], ilo_sb, BL * 128, BL * 128, D,
                    transpose=False, single_packet=False)
                ehv = emb_hi[:].rearrange("p (c e) -> p c e", e=D)
                nc.gpsimd.dma_gather(
                    ehv[:, 0:BL // 2, :],
                    ctable[:], ihi_sb[:, 0:128], (BL // 2) * 128,
                    (BL // 2) * 128, D, transpose=False, single_packet=False)
                nc.gpsimd.dma_gather(
                    ehv[:, BL // 2:BL, :],
                    ctable[:], ihi_sb[:, 128:256], (BL // 2) * 128,
                    (BL // 2) * 128, D, transpose=False, single_packet=False)

                # ---- edv = vm * ed; transpose to edvT[gc][d, 32j+b] ----
                edv_sb = sb.tile([128, D], F32, tag="edv")
                e4v_sb = sb.tile([BL, D], F32, tag="e4v")
                nc.vector.tensor_scalar_mul(out=edv_sb[:], in0=ed_sb[:],
                                            scalar1=vmf_sb)
                nc.vector.tensor_scalar_mul(out=e4v_sb[:], in0=e4_sb[:],
                                            scalar1=vm4_sb)
                edvT = [sb.tile([128, L * BL], BF16, tag=f"edvT{gc}",
                                name=f"edvT{gc}") for gc in range(2)]
                for gc in range(2):
                    tp = ps_t.tile([128, 320], F32, tag="ps_tr")
                    nc.tensor.transpose(out=tp[:, 0:128],
                                        in_=edv_sb[:, gc * 128:(gc + 1) * 128],
                                        identity=ident[:])
                    nc.vector.tensor_copy(out=edvT[gc][:, 0:128], in_=tp[:, 0:128])
                    tp4 = ps_t.tile([128, 320], F32, tag="ps_tr")
                    nc.tensor.transpose(out=tp4[:, 0:BL],
                                        in_=e4v_sb[:, gc * 128:(gc + 1) * 128],
                                        identity=ident[:BL, :BL])
                    nc.vector.tensor_copy(out=edvT[gc][:, 128:160],
                                          in_=tp4[:, 0:BL])

                # ---- qrawT[d,(l,b)] = cumsum_l edvT (DVE);
                #      q^T[e,(l,b)] = linT[l] @ qrawT[:, l] ----
                qrawT = [sb.tile([128, L * BL], BF16, tag=f"qrawT{gc}",
                                 name=f"qrawT{gc}") for gc in range(2)]
                for gc in range(2):
                    nc.vector.tensor_copy(out=qrawT[gc][:, 0:BL],
                                          in_=edvT[gc][:, 0:BL])
                    for j in range(1, L):
                        nc.vector.tensor_add(
                            out=qrawT[gc][:, j * BL:(j + 1) * BL],
                            in0=qrawT[gc][:, (j - 1) * BL:j * BL],
                            in1=edvT[gc][:, j * BL:(j + 1) * BL])
                lin_v = lin_sb.rearrange("p (l g c) -> p l g c", l=L, g=2)
                qT = [sb.tile([128, L * BL], BF16, tag=f"qT{eh}",
                              name=f"qT{eh}") for eh in range(2)]
                for eh in range(2):
                    qp = ps_t.tile([128, 320], F32, tag="ps_tr")
                    for l in range(L):
                        for gc in range(2):
                            nc.tensor.matmul(
                                out=qp[:, l * BL:(l + 1) * BL],
                                lhsT=lin_v[:, l, gc,
                                           eh * 128:(eh + 1) * 128],
                                rhs=qrawT[gc][:, l * BL:(l + 1) * BL],
                                start=(gc == 0), stop=(gc == 1),
                                skip_group_check=True)
                    nc.vector.tensor_copy(out=qT[eh][:], in_=qp[:, 0:L * BL])

                # ---- qt[f,(r,b)] = M_h^T-blocks @ qT; r = h*L+l padded to 32
                # r-cols 20..31 stay zero so score matmuls write full 32-row
                # aligned PSUM blocks (exp(0)=1 rows are killed by hsel=0).
                m_v = m_sb.rearrange("p (h e c) -> p h e c", h=H, e=2)
                qt = [sb.tile([128, 32 * BL], BF16, tag=f"qt{fh}",
                              name=f"qt{fh}") for fh in range(2)]
                for fh in range(2):
                    nc.vector.memset(qt[fh][:, NR * BL:32 * BL], 0.0)
                    for hp in range(2):
                        qtp = ps_t.tile([128, 320], F32, tag="ps_qt")
                        for hh in range(2):
                            h = hp * 2 + hh
                            for eh in range(2):
                                nc.tensor.matmul(
                                    out=qtp[:, hh * 160:(hh + 1) * 160],
                                    lhsT=m_v[:, h, eh, fh * 128:(fh + 1) * 128],
                                    rhs=qT[eh][:], start=(eh == 0),
                                    stop=(eh == 1), skip_group_check=True)
                        nc.vector.tensor_copy(
                            out=qt[fh][:, hp * 2 * L * BL:(hp + 1) * 2 * L * BL],
                            in_=qtp[:, 0:320])

                # ---- per-group: scores -> exp -> p4 pool -> final ----
                esum = sb.tile([128, NG], F32, tag="esum")
                nc.vector.memset(esum[:], 1.0)
                op = ps_op.tile([128, 2 * BL], F32, tag="ps_opt")  # [d%128,(dc,b)]

                def stage_scores(g):
                    spg = ps_sp.tile([128, S], F32, tag="ps_spg", name="spg")
                    for bb in range(GS):
                        b = g * GS + bb
                        k, bl = divmod(b, NH)
                        for fh in range(2):
                            qt_cols = qt[fh][:].rearrange(
                                "p (r b) -> p r b", r=32)[:, :, b]
                            nc.tensor.matmul(
                                out=spg[bb * 32:(bb + 1) * 32, 0:S],
                                lhsT=qt_cols,
                                rhs=embT[k][:].rearrange(
                                    "p (c i) -> p c i", c=2)[:, fh,
                                    bl * S:(bl + 1) * S],
                                start=(fh == 0), stop=(fh == 1),
                                tile_position=(0, bb * 32),
                                skip_group_check=True)
                    esc = sbg.tile([128, S], BF16, tag="esc", name=f"esc{g}")
                    nc.scalar.activation(
                        out=esc[:], in_=spg[:],
                        func=mybir.ActivationFunctionType.Exp)
                    nc.vector.reduce_sum(out=esum[:, g:g + 1], in_=esc[:],
                                         axis=mybir.AxisListType.X)
                    # attn^4 = esc^4 * (1/esum)^4
                    rinv = sbg.tile([128, 1], F32, tag="rinv", name="rinv")
                    nc.vector.reciprocal(out=rinv[:], in_=esum[:, g:g + 1])
                    nc.vector.tensor_mul(out=rinv[:], in0=rinv[:], in1=rinv[:])
                    nc.vector.tensor_mul(out=rinv[:], in0=rinv[:], in1=rinv[:])
                    nc.vector.tensor_mul(out=esc[:], in0=esc[:], in1=esc[:])
                    nc.vector.tensor_mul(out=esc[:], in0=esc[:], in1=esc[:])
                    esc4 = sbg.tile([128, S], BF16, tag="esc4", name=f"esc4_{g}")
                    nc.vector.tensor_scalar_mul(out=esc4[:], in0=esc[:],
                                                scalar1=rinv[:, 0:1])
                    return esc4

                def stage_pool(g, esc4):
                    p4t = ps_p4.tile([128, S + 2 * GS], F32, tag="ps_p4t",
                                     name="p4t")
                    nc.tensor.matmul(out=p4t[0:GS * H, 0:S], lhsT=hsel_v,
                                     rhs=esc4[:, :], start=True, stop=True,
                                     skip_group_check=True)
                    p4a = sbg.tile([GS * H, S], F32, tag="p4a", name=f"p4a{g}")
                    nc.scalar.activation(out=p4a[:], in_=p4t[0:GS * H, 0:S],
                                         func=mybir.ActivationFunctionType.Ln)
                    nc.scalar.activation(out=p4a[:], in_=p4a[:],
                                         func=mybir.ActivationFunctionType.Exp,
                                         scale=0.25)
                    nc.tensor.matmul(out=p4t[0:S0, S:S + GS], lhsT=p4a[:, 0:S0],
                                     rhs=bsel_v, start=True, stop=True,
                                     skip_group_check=True)
                    nc.tensor.matmul(out=p4t[0:S1, S + GS:S + 2 * GS],
                                     lhsT=p4a[:, S0:S],
                                     rhs=bsel_v, start=True, stop=True,
                                     skip_group_check=True)
                    poolT = sbg.tile([128, 2 * GS], BF16, tag="poolT",
                                     name=f"poolT{g}")
                    nc.vector.tensor_copy(out=poolT[0:S0, 0:GS],
                                          in_=p4t[0:S0, S:S + GS])
                    nc.vector.tensor_copy(out=poolT[0:S1, GS:2 * GS],
                                          in_=p4t[0:S1, S + GS:S + 2 * GS])
                    return poolT

                def stage_final(g, poolT):
                    for bb in range(GS):
                        b = g * GS + bb
                        for dc in range(2):
                            csl = slice(b * D + dc * 128, b * D + (dc + 1) * 128)
                            nc.tensor.matmul(out=op[:, dc * BL + b:dc * BL + b + 1],
                                             lhsT=emb_lo[:, csl],
                                             rhs=poolT[0:S0, bb:bb + 1],
                                             start=True, stop=False)
                            nc.tensor.matmul(out=op[:, dc * BL + b:dc * BL + b + 1],
                                             lhsT=emb_hi[0:S1, csl],
                                             rhs=poolT[0:S1, GS + bb:GS + bb + 1],
                                             start=False, stop=True)

                # software-pipelined: scores(g) | pool(g-1); all finals
                # at the end so the in-order PE stream never stalls on the
                # ACT chain or the late embS transfers
                esc4s, poolTs = {}, {}
                for gx in range(NG + 1):
                    if gx < NG:
                        esc4s[gx] = stage_scores(gx)
                    if gx >= 1:
                        poolTs[gx - 1] = stage_pool(gx - 1, esc4s.pop(gx - 1))
                for g in range(NG):
                    stage_final(g, poolTs.pop(g))

                # ---- store out^T [d%128, (dc, b)]; host untransposes ----
                outT = sb.tile([128, 2 * BL], F32, tag="outT")
                nc.vector.tensor_copy(out=outT[:], in_=op[:])
                nc.sync.dma_start(out=out[:], in_=outT[:])
    nc.compile()
    return nc


def _host_prep(item_seq, item_seq_len, emb_table, lin_W, WQ_W, WK_W):
    item_seq = np.asarray(item_seq).astype(np.int64)
    item_len = np.asarray(item_seq_len).astype(np.int64)
    emb_f = np.asarray(emb_table, dtype=np.float32)
    lin64 = np.asarray(lin_W, dtype=np.float64)
    WQ64 = np.asarray(WQ_W, dtype=np.float64)
    WK64 = np.asarray(WK_W, dtype=np.float64)

    # weights: linT[l] = lin_W[l].T  [d, e];  M[h] = WQ^T WK / sqrt(D)  [e, f]
    linT = np.transpose(lin64, (0, 2, 1))
    M = np.stack([WQ64[h].T @ WK64[h] for h in range(H)]) / np.sqrt(D)
    # lin_d rows ordered (l, gc, p=d%128), cols = e (full 256)
    lin_blk = np.zeros((L, 2, 128, 256), np.float64)
    m_blk = np.zeros((H, 2, 128, 256), np.float64)
    for l in range(L):
        for gc in range(2):
            lin_blk[l, gc] = linT[l][gc * 128:(gc + 1) * 128, :]
    for h in range(H):
        for eh in range(2):
            m_blk[h, eh] = M[h][eh * 128:(eh + 1) * 128, :]
    w_d = np.ascontiguousarray(np.concatenate(
        [lin_blk.reshape(L * 2 * 128, 256), m_blk.reshape(H * 2 * 128, 256)],
        axis=0).astype(ml_bf16))

    hsel = np.zeros((128, GS * H), np.float32)
    for bb in range(GS):
        for h in range(H):
            for l in range(L):
                hsel[bb * 32 + h * L + l, bb * H + h] = 1.0
    hselb = hsel.astype(ml_bf16)
    bsel = np.zeros((GS * H, GS), np.float32)
    for bb in range(GS):
        for h in range(H):
            bsel[bb * H + h, bb] = 0.25

    js = np.arange(L)[None, :]
    pos = np.maximum(item_len[:, None] - 1 - js, 0)          # [B, L]
    valid = (js <= item_len[:, None] - 1).astype(np.float32)  # [B, L]

    def wrap16(ids16, n):
        t = np.zeros((128, n // 16), np.int16)
        cols = np.arange(n) // 16
        rows = np.arange(n) % 16
        for gg in range(8):
            t[16 * gg + rows, cols] = ids16
        return t

    in_maps = []
    for c in range(NCORES):
        bsl = slice(c * BL, (c + 1) * BL)
        seq = item_seq[bsl]
        uni, inv = np.unique(seq, return_inverse=True)
        inv = inv.reshape(BL, S).astype(np.int64)
        ct = np.zeros((NT, D), ml_bf16)
        ct[:len(uni)] = emb_f[uni].astype(ml_bf16)

        # embT halves: i = (b % 16)*S + s
        itq_parts = []
        for k in range(2):
            ids_k = inv[k * NH:(k + 1) * NH].reshape(-1).astype(np.int16)
            itq_parts.append(wrap16(ids_k, NIT))
        idx_tq = np.concatenate(itq_parts, axis=1)
        # embS lo: i = b*128 + s (s < 128); hi: i = b*128 + (s-128), pad 0
        ids_lo = np.ascontiguousarray(inv[:, 0:128]).reshape(-1).astype(np.int16)
        hi = np.zeros((BL, 128), np.int64)
        hi[:, 0:S1] = inv[:, S0:S]
        ids_hi = hi.reshape(-1).astype(np.int16)

        posl = np.take_along_axis(inv, pos[bsl], axis=1)      # [BL, L] local
        ism = np.zeros((128, 8), np.int32)
        ism[0:GS * H, 4:8] = bsel.view(np.int32)
        for l in range(4):
            ism[32 * l:32 * (l + 1), 0] = posl[:, l]
        ism[0:BL, 1] = posl[:, 4]
        vmf = np.zeros(128, np.float32)
        for l in range(4):
            vmf[32 * l:32 * (l + 1)] = valid[bsl][:, l]
        ism[:, 2] = vmf.view(np.int32)
        vm4 = np.zeros(128, np.float32)
        vm4[0:BL] = valid[bsl][:, 4]
        ism[:, 3] = vm4.view(np.int32)
        ibg = np.concatenate([idx_tq, wrap16(ids_lo, BL * 128),
                              wrap16(ids_hi, BL * 128),
                              hselb.view(np.int16)], axis=1)

        in_maps.append({
            "ctable": ct,
            "idx_small": ism,
            "idx_big": np.ascontiguousarray(ibg),
            "w_d": w_d,
        })
    return in_maps


def _get_nc(reps=1):
    global _CACHED
    if _CACHED is None:
        _CACHED = {}
    if reps not in _CACHED:
        _CACHED[reps] = _build(reps)
    return _CACHED[reps]


def run_on_device(in_maps, trace=False):
    nc = _get_nc()
    return run_bass_kernel_spmd(nc, in_maps, list(range(NCORES)), trace=trace)


def kernel(item_seq, item_seq_len, emb_table, lin_W, WQ_W, WK_W):
    in_maps = _host_prep(item_seq, item_seq_len, emb_table, lin_W, WQ_W, WK_W)
    res = run_on_device(in_maps, trace=False)
    outs = []
    for c in range(NCORES):
        ot = np.asarray(res.results[c]["out"])          # [128, 2*BL]
        o = np.transpose(ot.reshape(128, 2, BL), (2, 1, 0)).reshape(BL, D)
        outs.append(o)
    return np.concatenate(outs, axis=0)
